# revision 21
# baseline (speedup 1.0000x reference)
"""Trainium2 Bass kernel for a 4-layer post-LN transformer encoder.

Sharding: sequence-parallel. 8 cores = 2 batch groups x 4 sequence slices of
512 tokens. Per layer each core computes K/V for its own tokens, AllGathers
K/V within its 4-core batch group, and runs attention for its 512 queries
over all 2048 keys.

Layout: activations are kept feature-major (feature on partitions, tokens on
the free axis) so every matmul uses weight tiles as the stationary operand
with a 512-wide moving dim. Softmax and LayerNorm reductions (over the
partition axis) are done with ones-matmuls on the PE; exp/ln run on the
scalar engine (single table set). Matmuls use the fp32r datapath (full PE
speed, ~1e-4 rounding).

Self-contained: shapes/sharding hardcoded from the problem spec.
"""
import numpy as np
import ml_dtypes

import concourse.bass as bass
import concourse.mybir as mybir
import concourse.tile as tile
from concourse import bacc
from concourse.bass_utils import run_bass_kernel_spmd
from concourse.masks import make_identity

V, D, L, H, F, MAXLEN = 32000, 512, 4, 8, 2048, 2048
B, S = 2, 2048
NC = 8
GPS = 4          # cores per batch group
T = S // GPS     # 512 local tokens per core
P = 128
NT = T // P      # 4 local token tiles
DD = D // P      # 4 feature tiles
KTN = S // P     # 16 key tiles
FFN = F // P     # 16 mlp hidden tiles
PAIRS = H // 2   # 4 head pairs (2 heads = 128 features)
EPS = 1e-6

f32 = mybir.dt.float32
f32r = mybir.dt.float32r
bf16 = mybir.dt.bfloat16
MMDT = bf16
i32 = mybir.dt.int32
AF = mybir.ActivationFunctionType
ALU = mybir.AluOpType
GROUPS = [[0, 1, 2, 3], [4, 5, 6, 7]]

ts = bass.ts


def _layer_norm(nc, pools, v_tiles, g_t, be_t, ones128, eps_t, out_pool, out_tag):
    """Feature-axis layernorm on 4 feature-major (128, T) f32r tiles.

    Returns 4 new f32r tiles from out_pool with tag out_tag.
    """
    st, sb4 = pools["st"], pools["sb4"]
    s1 = st.tile([P, T], f32, tag="st")
    s2 = st.tile([P, T], f32, tag="st")
    sq_tiles = []
    for dd in range(DD):
        sq = sb4.tile([P, T], f32r, tag="tmp")
        nc.vector.tensor_tensor(
            out=sq[:], in0=v_tiles[dd][:].bitcast(f32),
            in1=v_tiles[dd][:].bitcast(f32), op=ALU.mult)
        sq_tiles.append(sq)
    for dd in range(DD):
        nc.tensor.matmul(out=s1[:], lhsT=ones128[:], rhs=v_tiles[dd][:],
                         start=(dd == 0), stop=(dd == DD - 1))
    for dd in range(DD):
        nc.tensor.matmul(out=s2[:], lhsT=ones128[:], rhs=sq_tiles[dd][:],
                         start=(dd == 0), stop=(dd == DD - 1))
    # mean (broadcast over partitions), and 512*var = S2 - S1^2/512
    mean_b = sb4.tile([P, T], f32, tag="lns")
    nc.vector.tensor_scalar(out=mean_b[:], in0=s1[:], scalar1=1.0 / D,
                            scalar2=None, op0=ALU.mult)
    s1s = sb4.tile([P, T], f32, tag="lns")
    nc.vector.tensor_scalar(out=s1s[:], in0=s1[:], scalar1=1.0 / float(np.sqrt(D)),
                            scalar2=None, op0=ALU.mult)
    msq = sb4.tile([P, T], f32, tag="lns")
    nc.vector.tensor_tensor(out=msq[:], in0=s1s[:], in1=s1s[:], op=ALU.mult)
    varx = sb4.tile([P, T], f32, tag="lns")
    nc.vector.tensor_tensor(out=varx[:], in0=s2[:], in1=msq[:], op=ALU.subtract)
    # rstd = exp(-0.5 * ln(varx/512 + eps)) ; broadcast tile
    lnv = sb4.tile([P, T], f32, tag="lns")
    nc.scalar.activation(out=lnv[:], in_=varx[:], func=AF.Ln,
                         scale=1.0 / D, bias=eps_t[:, :1])
    rstd = sb4.tile([P, T], f32, tag="lns")
    nc.scalar.activation(out=rstd[:], in_=lnv[:], func=AF.Exp, scale=-0.5)

    out_tiles = []
    for dd in range(DD):
        d1 = sb4.tile([P, T], f32, tag="tmp")
        nc.vector.tensor_tensor(out=d1[:], in0=v_tiles[dd][:].bitcast(f32),
                                in1=mean_b[:], op=ALU.subtract)
        d2 = sb4.tile([P, T], f32, tag="tmp")
        nc.vector.tensor_tensor(out=d2[:], in0=d1[:], in1=rstd[:], op=ALU.mult)
        o = out_pool.tile([P, T], f32r, tag=out_tag)
        nc.vector.tensor_scalar(out=o[:], in0=d2[:],
                                scalar1=g_t[:, dd:dd + 1],
                                scalar2=be_t[:, dd:dd + 1],
                                op0=ALU.mult, op1=ALU.add)
        out_tiles.append(o)
    return out_tiles


def build_encoder(reps=1, no_collective=False):
    nc = bacc.Bacc("TRN2", target_bir_lowering=False, debug=False,
                   num_devices=NC)

    x_idx = nc.dram_tensor("x_idx", [T, 1], i32, kind="ExternalInput")
    pos_idx = nc.dram_tensor("pos_idx", [T, 1], i32, kind="ExternalInput")
    tok_emb = nc.dram_tensor("tok_emb", [V, D], f32, kind="ExternalInput")
    sin_table = nc.dram_tensor("sin_table", [MAXLEN + 3, D], f32, kind="ExternalInput")
    Wq = nc.dram_tensor("Wq", [L, D, D], f32, kind="ExternalInput")
    Wk = nc.dram_tensor("Wk", [L, D, D], f32, kind="ExternalInput")
    Wv = nc.dram_tensor("Wv", [L, D, D], f32, kind="ExternalInput")
    Wo = nc.dram_tensor("Wo", [L, D, D], f32, kind="ExternalInput")
    bq = nc.dram_tensor("bq", [L, D], f32, kind="ExternalInput")
    bk = nc.dram_tensor("bk", [L, D], f32, kind="ExternalInput")
    bv = nc.dram_tensor("bv", [L, D], f32, kind="ExternalInput")
    bo = nc.dram_tensor("bo", [L, D], f32, kind="ExternalInput")
    W1 = nc.dram_tensor("W1", [L, D, F], f32, kind="ExternalInput")
    b1 = nc.dram_tensor("b1", [L, F], f32, kind="ExternalInput")
    W2 = nc.dram_tensor("W2", [L, F, D], f32, kind="ExternalInput")
    b2 = nc.dram_tensor("b2", [L, D], f32, kind="ExternalInput")
    g1 = nc.dram_tensor("g1", [L, D], f32, kind="ExternalInput")
    be1 = nc.dram_tensor("be1", [L, D], f32, kind="ExternalInput")
    g2 = nc.dram_tensor("g2", [L, D], f32, kind="ExternalInput")
    be2 = nc.dram_tensor("be2", [L, D], f32, kind="ExternalInput")

    out_h = nc.dram_tensor("out_h", [T, D], f32, kind="ExternalOutput")

    KW = H * 65
    kv_in = [nc.dram_tensor(f"kv_in_{li}", [2 * T, KW], MMDT) for li in range(L)]
    kv_out = [nc.dram_tensor(f"kv_out_{li}", [GPS * 2 * T, KW], MMDT)
              for li in range(L)]

    with tile.TileContext(nc) as tc:
        with (
            tc.tile_pool(name="consts", bufs=1) as consts,
            tc.tile_pool(name="hxp", bufs=8) as hxp,
        ):
            ident = consts.tile([P, P], f32)
            make_identity(nc, ident[:])
            ones_bf = consts.tile([P, P], MMDT)
            nc.vector.memset(ones_bf[:], 1.0)
            ones_f = consts.tile([P, P], f32)
            nc.vector.memset(ones_f[:], 1.0)
            ones128 = consts.tile([P, P], f32r)
            nc.vector.tensor_copy(out=ones128[:], in_=ones_f[:])
            eps_t = consts.tile([P, 1], f32)
            nc.vector.memset(eps_t[:], EPS)

            for _rep in range(reps):
                # ================= embedding =================
                hx = []
                with (
                    tc.tile_pool(name="emb", bufs=2) as emb,
                    tc.tile_pool(name="emb_ps", bufs=4, space="PSUM") as emb_ps,
                ):
                    idx_t = emb.tile([P, NT, 1], i32, tag="idx")
                    nc.sync.dma_start(
                        out=idx_t[:],
                        in_=x_idx[:].rearrange("(j p) o -> p j o", p=P))
                    pid_t = emb.tile([P, NT, 1], i32, tag="idx")
                    nc.sync.dma_start(
                        out=pid_t[:],
                        in_=pos_idx[:].rearrange("(j p) o -> p j o", p=P))
                    for dd in range(DD):
                        hx.append(hxp.tile([P, T], f32r, tag="hx", name=f"hx{dd}"))
                    for j in range(NT):
                        tok_g = emb.tile([P, D], f32, tag="tok")
                        nc.gpsimd.indirect_dma_start(
                            out=tok_g[:], out_offset=None, in_=tok_emb[:],
                            in_offset=bass.IndirectOffsetOnAxis(
                                ap=idx_t[:, j, :], axis=0))
                        pos_g = emb.tile([P, D], f32, tag="pos")
                        nc.gpsimd.indirect_dma_start(
                            out=pos_g[:], out_offset=None, in_=sin_table[:],
                            in_offset=bass.IndirectOffsetOnAxis(
                                ap=pid_t[:, j, :], axis=0))
                        h0 = emb.tile([P, D], f32, tag="h0")
                        nc.vector.tensor_tensor(out=h0[:], in0=tok_g[:],
                                                in1=pos_g[:], op=ALU.add)
                        for dd in range(DD):
                            tp = emb_ps.tile([P, P], f32, tag="tr")
                            nc.tensor.transpose(out=tp[:],
                                                in_=h0[:, ts(dd, P)],
                                                identity=ident[:])
                            nc.vector.tensor_copy(out=hx[dd][:, ts(j, P)],
                                                  in_=tp[:])

                # ================= layers =================
                for li in range(L):
                    with (
                        tc.tile_pool(name="sbW", bufs=2) as sbW,
                        tc.tile_pool(name="sbW3", bufs=3) as sbW3,
                        tc.tile_pool(name="sb4", bufs=4) as sb4,
                        tc.tile_pool(name="mtp", bufs=5) as mtp,
                        tc.tile_pool(name="bias", bufs=1) as bias,
                    ):
                        # --- biases / gains for this layer ---
                        bq_t = bias.tile([P, DD], f32, tag="bq")
                        nc.sync.dma_start(out=bq_t[:], in_=bq[li].rearrange("(a p) -> p a", p=P))
                        bk_t = bias.tile([P, DD], f32, tag="bk")
                        nc.sync.dma_start(out=bk_t[:], in_=bk[li].rearrange("(a p) -> p a", p=P))
                        bo_t = bias.tile([P, DD], f32, tag="bo")
                        nc.sync.dma_start(out=bo_t[:], in_=bo[li].rearrange("(a p) -> p a", p=P))
                        b2_t = bias.tile([P, DD], f32, tag="b2")
                        nc.sync.dma_start(out=b2_t[:], in_=b2[li].rearrange("(a p) -> p a", p=P))
                        b1_t = bias.tile([P, FFN], f32, tag="b1")
                        nc.sync.dma_start(out=b1_t[:], in_=b1[li].rearrange("(a p) -> p a", p=P))
                        g1_t = bias.tile([P, DD], f32, tag="g1")
                        nc.sync.dma_start(out=g1_t[:], in_=g1[li].rearrange("(a p) -> p a", p=P))
                        be1_t = bias.tile([P, DD], f32, tag="be1")
                        nc.sync.dma_start(out=be1_t[:], in_=be1[li].rearrange("(a p) -> p a", p=P))
                        g2_t = bias.tile([P, DD], f32, tag="g2")
                        nc.sync.dma_start(out=g2_t[:], in_=g2[li].rearrange("(a p) -> p a", p=P))
                        be2_t = bias.tile([P, DD], f32, tag="be2")
                        nc.sync.dma_start(out=be2_t[:], in_=be2[li].rearrange("(a p) -> p a", p=P))
                        bv_row = bias.tile([1, D], f32, tag="bvr")
                        nc.sync.dma_start(out=bv_row[:], in_=bv[li][None, :])
                        bv_row_r = bias.tile([1, D], MMDT, tag="bvrr")
                        nc.vector.tensor_copy(out=bv_row_r[:], in_=bv_row[:])

                        def load_w_r(w_ap, shape, tag, name="wr", eng=None):
                            stg = sbW3.tile(shape, f32, tag="wstage", name=name + "_stg")
                            nc.sync.dma_start(out=stg[:], in_=w_ap)
                            wr = sbW.tile(shape, f32r, tag=tag, name=name)
                            if eng == "act":
                                nc.scalar.copy(out=wr[:], in_=stg[:])
                            else:
                                nc.vector.tensor_copy(out=wr[:], in_=stg[:])
                            return wr

                        # ---------- phase A: Q/K/V projections ----------
                        with tc.tile_pool(name="psA", bufs=4, space="PSUM") as psA:
                            wk_r = load_w_r(Wk[li].rearrange("(a p) m -> p a m", p=P),
                                            [P, DD, D], "wproj", eng="act")
                            k_sb = []
                            for dd in range(DD):
                                ps = psA.tile([P, T], f32, tag="pj")
                                for a in range(DD):
                                    nc.tensor.matmul(
                                        out=ps[:], lhsT=wk_r[:, a, ts(dd, P)],
                                        rhs=hx[a][:], start=(a == 0),
                                        stop=(a == DD - 1))
                                kt_ = sb4.tile([P, T], MMDT, tag="k")
                                nc.vector.tensor_scalar(
                                    out=kt_[:], in0=ps[:],
                                    scalar1=bk_t[:, dd:dd + 1], scalar2=None,
                                    op0=ALU.add)
                                k_sb.append(kt_)
                                nc.sync.dma_start(
                                    out=kv_in[li][ts(dd, P), 0:D],
                                    in_=kt_[:])
                            # V token-major: stationary hx tile, moving Wv rows
                            wv_r = load_w_r(Wv[li].rearrange("(a p) m -> p a m", p=P),
                                            [P, DD, D], "wproj", eng="act")
                            # bv broadcast tile
                            bvb_ps = psA.tile([P, D], f32, tag="pj")
                            nc.tensor.matmul(out=bvb_ps[:],
                                             lhsT=ones_bf[0:1, :],
                                             rhs=bv_row_r[:],
                                             start=True, stop=True)
                            bvb = bias.tile([P, D], f32, tag="bvb")
                            nc.vector.tensor_copy(out=bvb[:], in_=bvb_ps[:])
                            for j in range(NT):
                                ps = psA.tile([P, D], f32, tag="pj")
                                for a in range(DD):
                                    nc.tensor.matmul(
                                        out=ps[:], lhsT=hx[a][:, ts(j, P)],
                                        rhs=wv_r[:, a, :], start=(a == 0),
                                        stop=(a == DD - 1))
                                vt_ = sb4.tile([P, H, 65], MMDT, tag="v")
                                nc.vector.tensor_tensor(
                                    out=vt_[:, :, 0:64],
                                    in0=ps[:].rearrange("p (h c) -> p h c", h=H),
                                    in1=bvb[:].rearrange("p (h c) -> p h c", h=H),
                                    op=ALU.add)
                                nc.vector.memset(vt_[:, :, 64:65], 1.0)
                                nc.sync.dma_start(
                                    out=kv_in[li][T + j * P:T + (j + 1) * P, :],
                                    in_=vt_[:].rearrange("p h c -> p (h c)"))
                            if no_collective:
                                # timing-only variant: replicate own K/V
                                for r in range(GPS):
                                    nc.sync.dma_start(
                                        out=kv_out[li][r * 2 * T:(r + 1) * 2 * T, :],
                                        in_=kv_in[li][:])
                            else:
                                nc.gpsimd.collective_compute(
                                    "AllGather", ALU.bypass,
                                    replica_groups=GROUPS,
                                    ins=[kv_in[li][:]], outs=[kv_out[li][:]])
                            # Q after the AG is kicked off
                            wq_r = load_w_r(Wq[li].rearrange("(a p) m -> p a m", p=P),
                                            [P, DD, D], "wproj", eng="act")
                            q_sb = []
                            for dd in range(DD):
                                ps = psA.tile([P, T], f32, tag="pj")
                                for a in range(DD):
                                    nc.tensor.matmul(
                                        out=ps[:], lhsT=wq_r[:, a, ts(dd, P)],
                                        rhs=hx[a][:], start=(a == 0),
                                        stop=(a == DD - 1))
                                qt_ = sb4.tile([P, T], MMDT, tag="q")
                                nc.vector.tensor_scalar(
                                    out=qt_[:], in0=ps[:],
                                    scalar1=bq_t[:, dd:dd + 1], scalar2=None,
                                    op0=ALU.add)
                                q_sb.append(qt_)
                            wo_view = Wo[li].rearrange(
                                "(c hh p) m -> c p hh m", c=2, hh=4, p=64)
                            wo_c = []
                            for c in range(2):
                                woc = load_w_r(wo_view[c], [64, 4, D], "wproj",
                                               name=f"wo{c}")
                                wo_c.append(woc)

                        # views of the gathered K/V
                        kv_k_view = kv_out[li][:].rearrange(
                            "(r a pp p) n -> a pp p r n", r=GPS, a=2, pp=DD, p=P)
                        kv_v_view = kv_out[li][:].rearrange(
                            "(r a j p) n -> a p r j n", r=GPS, a=2, j=NT, p=P)

                        # ---------- phase B: attention ----------
                        an_sb = []
                        with tc.tile_pool(name="psB", bufs=2, space="PSUM") as psB:
                            for pp in range(PAIRS):
                                kf = sbW.tile([P, GPS, T], MMDT, tag="kfr")
                                nc.sync.dma_start(out=kf[:],
                                                  in_=kv_k_view[0, pp][:, :, 0:D])
                                kff = kf[:].rearrange("p r n -> p (r n)")
                                # V for the two heads of this pair, each
                                # augmented with a ones column: the att matmul
                                # then also produces the softmax denominator
                                # in output row 64.
                                vf = sbW.tile([P, GPS, NT, 130], MMDT, tag="vfr")
                                for r in range(GPS):
                                    nc.sync.dma_start(
                                        out=vf[:, r],
                                        in_=kv_v_view[1, :, r, :,
                                                      pp * 130:(pp + 1) * 130])
                                vff = vf[:].rearrange("p r j c -> p (r j) c")

                                att = [psB.tile([65, T], f32, tag="att",
                                                name=f"att{pp}_{hh}")
                                       for hh in range(2)]
                                for kt in range(KTN):
                                    sg = psB.tile([P, 2, T], f32, tag="sc")
                                    nc.tensor.matmul(
                                        out=sg[:, 0, :],
                                        lhsT=kff[0:64, ts(kt, P)],
                                        rhs=q_sb[pp][0:64, :],
                                        start=True, stop=True,
                                        tile_position=(0, 0))
                                    nc.tensor.matmul(
                                        out=sg[:, 1, :],
                                        lhsT=kff[64:128, ts(kt, P)],
                                        rhs=q_sb[pp][64:128, :],
                                        start=True, stop=True,
                                        tile_position=(64, 0))
                                    e_t = sbW3.tile([P, 2, T], MMDT, tag="e")
                                    nc.scalar.activation(out=e_t[:], in_=sg[:],
                                                         func=AF.Exp, scale=0.125)
                                    first, last = kt == 0, kt == KTN - 1
                                    for hh in range(2):
                                        nc.tensor.matmul(
                                            out=att[hh][:],
                                            lhsT=vff[:, kt, hh * 65:(hh + 1) * 65],
                                            rhs=e_t[:, hh, :], start=first,
                                            stop=last)
                                for hh in range(2):
                                    rc1 = sbW.tile([65, T], f32, tag="rc")
                                    nc.scalar.activation(
                                        out=rc1[64:65, :], in_=att[hh][64:65, :],
                                        func=AF.Ln)
                                    rc2 = sbW.tile([65, T], MMDT, tag="rc")
                                    nc.scalar.activation(
                                        out=rc2[64:65, :], in_=rc1[64:65, :],
                                        func=AF.Exp, scale=-1.0)
                                    rb_ps = psB.tile([64, T], f32, tag="rb")
                                    nc.tensor.matmul(
                                        out=rb_ps[:],
                                        lhsT=ones_bf[64:65, 0:64],
                                        rhs=rc2[64:65, :],
                                        start=True, stop=True)
                                    rb_sb = sbW.tile([64, T], f32, tag="rb_sb")
                                    nc.vector.tensor_copy(out=rb_sb[:], in_=rb_ps[:])
                                    an = hxp.tile([64, T], f32r, tag="an",
                                                  name=f"an{pp}_{hh}")
                                    nc.vector.tensor_tensor(
                                        out=an[:], in0=att[hh][0:64, :],
                                        in1=rb_sb[:], op=ALU.mult)
                                    an_sb.append(an)

                        # ---------- phase C: O-proj, LN1, MLP, LN2 ----------
                        with (
                            tc.tile_pool(name="psC", bufs=2, space="PSUM") as psC,
                            tc.tile_pool(name="psY", bufs=4, space="PSUM") as psY,
                            tc.tile_pool(name="stp", bufs=2, space="PSUM") as stp,
                        ):
                            pools = {"st": stp, "sb4": sb4}
                            v1 = []
                            for dd in range(DD):
                                ps = psC.tile([P, T], f32, tag="mm1")
                                for h in range(H):
                                    c, hh = divmod(h, 4)
                                    nc.tensor.matmul(
                                        out=ps[:], lhsT=wo_c[c][:, hh, ts(dd, P)],
                                        rhs=an_sb[h][:], start=(h == 0),
                                        stop=(h == H - 1))
                                of = sbW.tile([P, T], f32, tag="otmp")
                                nc.vector.tensor_scalar(
                                    out=of[:], in0=ps[:],
                                    scalar1=bo_t[:, dd:dd + 1], scalar2=None,
                                    op0=ALU.add)
                                vv = sb4.tile([P, T], f32r, tag="vres")
                                nc.vector.tensor_tensor(
                                    out=vv[:], in0=of[:],
                                    in1=hx[dd][:].bitcast(f32), op=ALU.add)
                                v1.append(vv)
                            hmid = _layer_norm(nc, pools, v1, g1_t, be1_t,
                                               ones128, eps_t, sb4, "hmid")

                            # MLP in quarters of F
                            y_ps = [psY.tile([P, T], f32, tag="y", name=f"y{dd}")
                                    for dd in range(DD)]
                            for qf in range(4):
                                w1q = load_w_r(
                                    W1[li].rearrange("(a p) f -> p a f", p=P)[
                                        :, :, qf * 512:(qf + 1) * 512],
                                    [P, DD, 512], "wmlp", eng="act")
                                mts = []
                                for ffq in range(4):
                                    ff = qf * 4 + ffq
                                    ps = psC.tile([P, T], f32, tag="mm1")
                                    for a in range(DD):
                                        nc.tensor.matmul(
                                            out=ps[:],
                                            lhsT=w1q[:, a, ts(ffq, P)],
                                            rhs=hmid[a][:], start=(a == 0),
                                            stop=(a == DD - 1))
                                    mt = mtp.tile([P, T], f32r, tag="mt")
                                    nc.vector.tensor_scalar(
                                        out=mt[:], in0=ps[:],
                                        scalar1=b1_t[:, ff:ff + 1],
                                        scalar2=0.0, op0=ALU.add, op1=ALU.max)
                                    mts.append(mt)
                                w2q = load_w_r(
                                    W2[li].rearrange("(a p) m -> p a m", p=P)[
                                        :, qf * 4:(qf + 1) * 4, :],
                                    [P, 4, D], "wmlp", eng="act")
                                for dd in range(DD):
                                    for ffq in range(4):
                                        nc.tensor.matmul(
                                            out=y_ps[dd][:],
                                            lhsT=w2q[:, ffq, ts(dd, P)],
                                            rhs=mts[ffq][:],
                                            start=(qf == 0 and ffq == 0),
                                            stop=(qf == 3 and ffq == 3))
                            v2 = []
                            for dd in range(DD):
                                yf = sbW.tile([P, T], f32, tag="otmp")
                                nc.vector.tensor_scalar(
                                    out=yf[:], in0=y_ps[dd][:],
                                    scalar1=b2_t[:, dd:dd + 1], scalar2=None,
                                    op0=ALU.add)
                                vv = sb4.tile([P, T], f32r, tag="vres")
                                nc.vector.tensor_tensor(
                                    out=vv[:], in0=yf[:],
                                    in1=hmid[dd][:].bitcast(f32), op=ALU.add)
                                v2.append(vv)
                            hx = _layer_norm(nc, pools, v2, g2_t, be2_t,
                                             ones128, eps_t, hxp, "hx")

                # ================= output transpose =================
                with (
                    tc.tile_pool(name="fin", bufs=2) as fin,
                    tc.tile_pool(name="fin_ps", bufs=4, space="PSUM") as fin_ps,
                ):
                    for j in range(NT):
                        ot = fin.tile([P, D], f32, tag="ot")
                        for dd in range(DD):
                            tp = fin_ps.tile([P, P], f32, tag="tr")
                            nc.tensor.transpose(
                                out=tp[:], in_=hx[dd][:, ts(j, P)].bitcast(f32),
                                identity=ident[:])
                            nc.vector.tensor_copy(out=ot[:, ts(dd, P)], in_=tp[:])
                        nc.sync.dma_start(out=out_h[ts(j, P), :], in_=ot[:])

    nc.compile()
    return nc


_NC_CACHE = {}


def _get_nc(reps=1, no_collective=False):
    key = (reps, no_collective)
    if key not in _NC_CACHE:
        _NC_CACHE[key] = build_encoder(reps, no_collective)
    return _NC_CACHE[key]


def make_in_maps(x, tok_emb, sin_table, Wq, bq, Wk, bk, Wv, bv, Wo, bo,
                 W1, b1, W2, b2, g1, be1, g2, be2):
    x = np.asarray(x)
    shared = dict(
        tok_emb=np.ascontiguousarray(tok_emb, np.float32),
        sin_table=np.ascontiguousarray(sin_table, np.float32),
        Wq=np.ascontiguousarray(Wq, np.float32),
        Wk=np.ascontiguousarray(Wk, np.float32),
        Wv=np.ascontiguousarray(Wv, np.float32),
        Wo=np.ascontiguousarray(Wo, np.float32),
        bq=np.ascontiguousarray(bq, np.float32),
        bk=np.ascontiguousarray(bk, np.float32),
        bv=np.ascontiguousarray(bv, np.float32),
        bo=np.ascontiguousarray(bo, np.float32),
        W1=np.ascontiguousarray(W1, np.float32),
        b1=np.ascontiguousarray(b1, np.float32),
        W2=np.ascontiguousarray(W2, np.float32),
        b2=np.ascontiguousarray(b2, np.float32),
        g1=np.ascontiguousarray(g1, np.float32),
        be1=np.ascontiguousarray(be1, np.float32),
        g2=np.ascontiguousarray(g2, np.float32),
        be2=np.ascontiguousarray(be2, np.float32),
    )
    in_maps = []
    for c in range(NC):
        b, sl = divmod(c, GPS)
        xs = np.ascontiguousarray(x[b, sl * T:(sl + 1) * T].astype(np.int32))
        pos = ((np.arange(sl * T, (sl + 1) * T, dtype=np.int64) + 1)
               * (xs != 0)).astype(np.int32)
        m = dict(shared)
        m["x_idx"] = xs.reshape(T, 1)
        m["pos_idx"] = pos.reshape(T, 1)
        in_maps.append(m)
    return in_maps


def kernel(x, mask, tok_emb, sin_table, Wq, bq, Wk, bk, Wv, bv, Wo, bo,
           W1, b1, W2, b2, g1, be1, g2, be2):
    # mask is all-ones per the problem spec (fill="ones"); softmax with an
    # all-true mask is plain softmax, so it is not shipped to the device.
    nc = _get_nc(1)
    in_maps = make_in_maps(x, tok_emb, sin_table, Wq, bq, Wk, bk, Wv, bv,
                           Wo, bo, W1, b1, W2, b2, g1, be1, g2, be2)
    res = run_bass_kernel_spmd(nc, in_maps, core_ids=list(range(NC)))
    out = np.zeros((B, S, D), np.float32)
    for c in range(NC):
        b, sl = divmod(c, GPS)
        out[b, sl * T:(sl + 1) * T, :] = res.results[c]["out_h"]
    return out


# revision 22
# speedup vs baseline: 1.4836x; 1.4836x over previous
"""Trainium2 Bass kernel for a 4-layer post-LN transformer encoder.

Sharding: sequence-parallel. 8 cores = 2 batch groups x 4 sequence slices of
512 tokens. Per layer each core computes K/V for its own tokens, AllGathers
K/V within its 4-core batch group, and runs attention for its 512 queries
over all 2048 keys.

Layout: activations are kept feature-major (feature on partitions, tokens on
the free axis) so every matmul uses weight tiles as the stationary operand
with a 512-wide moving dim. Softmax and LayerNorm reductions (over the
partition axis) are done with ones-matmuls on the PE; exp/ln run on the
scalar engine (single table set). Matmuls use the fp32r datapath (full PE
speed, ~1e-4 rounding).

Self-contained: shapes/sharding hardcoded from the problem spec.
"""
import numpy as np
import ml_dtypes

import concourse.bass as bass
import concourse.mybir as mybir
import concourse.tile as tile
from concourse import bacc
from concourse.bass_utils import run_bass_kernel_spmd
from concourse.masks import make_identity

V, D, L, H, F, MAXLEN = 32000, 512, 4, 8, 2048, 2048
B, S = 2, 2048
NC = 8
GPS = 4          # cores per batch group
T = S // GPS     # 512 local tokens per core
P = 128
NT = T // P      # 4 local token tiles
DD = D // P      # 4 feature tiles
KTN = S // P     # 16 key tiles
FFN = F // P     # 16 mlp hidden tiles
PAIRS = H // 2   # 4 head pairs (2 heads = 128 features)
EPS = 1e-6

f32 = mybir.dt.float32
f32r = mybir.dt.float32r
bf16 = mybir.dt.bfloat16
MMDT = bf16
i32 = mybir.dt.int32
AF = mybir.ActivationFunctionType
ALU = mybir.AluOpType
GROUPS = [[0, 1, 2, 3], [4, 5, 6, 7]]

ts = bass.ts


def _layer_norm(nc, pools, v_tiles, g_t, be_t, ones128, eps_t, out_pool, out_tag):
    """Feature-axis layernorm on 4 feature-major (128, T) f32r tiles.

    Returns 4 new f32r tiles from out_pool with tag out_tag.
    """
    st, sb4 = pools["st"], pools["sb4"]
    s1 = st.tile([P, T], f32, tag="st")
    s2 = st.tile([P, T], f32, tag="st")
    sq_tiles = []
    for dd in range(DD):
        sq = sb4.tile([P, T], f32r, tag="tmp")
        nc.vector.tensor_tensor(
            out=sq[:], in0=v_tiles[dd][:].bitcast(f32),
            in1=v_tiles[dd][:].bitcast(f32), op=ALU.mult)
        sq_tiles.append(sq)
    for dd in range(DD):
        nc.tensor.matmul(out=s1[:], lhsT=ones128[:], rhs=v_tiles[dd][:],
                         start=(dd == 0), stop=(dd == DD - 1))
    for dd in range(DD):
        nc.tensor.matmul(out=s2[:], lhsT=ones128[:], rhs=sq_tiles[dd][:],
                         start=(dd == 0), stop=(dd == DD - 1))
    # mean (broadcast over partitions), and 512*var = S2 - S1^2/512
    mean_b = sb4.tile([P, T], f32, tag="lns")
    nc.vector.tensor_scalar(out=mean_b[:], in0=s1[:], scalar1=1.0 / D,
                            scalar2=None, op0=ALU.mult)
    s1s = sb4.tile([P, T], f32, tag="lns")
    nc.vector.tensor_scalar(out=s1s[:], in0=s1[:], scalar1=1.0 / float(np.sqrt(D)),
                            scalar2=None, op0=ALU.mult)
    msq = sb4.tile([P, T], f32, tag="lns")
    nc.vector.tensor_tensor(out=msq[:], in0=s1s[:], in1=s1s[:], op=ALU.mult)
    varx = sb4.tile([P, T], f32, tag="lns")
    nc.vector.tensor_tensor(out=varx[:], in0=s2[:], in1=msq[:], op=ALU.subtract)
    # rstd = exp(-0.5 * ln(varx/512 + eps)) ; broadcast tile
    lnv = sb4.tile([P, T], f32, tag="lns")
    nc.scalar.activation(out=lnv[:], in_=varx[:], func=AF.Ln,
                         scale=1.0 / D, bias=eps_t[:, :1])
    rstd = sb4.tile([P, T], f32, tag="lns")
    nc.scalar.activation(out=rstd[:], in_=lnv[:], func=AF.Exp, scale=-0.5)

    out_tiles = []
    for dd in range(DD):
        d1 = sb4.tile([P, T], f32, tag="tmp")
        nc.vector.tensor_tensor(out=d1[:], in0=v_tiles[dd][:].bitcast(f32),
                                in1=mean_b[:], op=ALU.subtract)
        d2 = sb4.tile([P, T], f32, tag="tmp")
        nc.vector.tensor_tensor(out=d2[:], in0=d1[:], in1=rstd[:], op=ALU.mult)
        o = out_pool.tile([P, T], f32r, tag=out_tag)
        nc.vector.tensor_scalar(out=o[:], in0=d2[:],
                                scalar1=g_t[:, dd:dd + 1],
                                scalar2=be_t[:, dd:dd + 1],
                                op0=ALU.mult, op1=ALU.add)
        out_tiles.append(o)
    return out_tiles


def build_encoder(reps=1, no_collective=False):
    nc = bacc.Bacc("TRN2", target_bir_lowering=False, debug=False,
                   num_devices=NC)

    x_idx = nc.dram_tensor("x_idx", [T, 1], i32, kind="ExternalInput")
    pos_idx = nc.dram_tensor("pos_idx", [T, 1], i32, kind="ExternalInput")
    tok_emb = nc.dram_tensor("tok_emb", [V, D], f32, kind="ExternalInput")
    sin_table = nc.dram_tensor("sin_table", [MAXLEN + 3, D], f32, kind="ExternalInput")
    Wq = nc.dram_tensor("Wq", [L, D, D], f32, kind="ExternalInput")
    Wk = nc.dram_tensor("Wk", [L, D, D], f32, kind="ExternalInput")
    Wv = nc.dram_tensor("Wv", [L, D, D], f32, kind="ExternalInput")
    Wo = nc.dram_tensor("Wo", [L, D, D], f32, kind="ExternalInput")
    bq = nc.dram_tensor("bq", [L, D], f32, kind="ExternalInput")
    bk = nc.dram_tensor("bk", [L, D], f32, kind="ExternalInput")
    bv = nc.dram_tensor("bv", [L, D], f32, kind="ExternalInput")
    bo = nc.dram_tensor("bo", [L, D], f32, kind="ExternalInput")
    W1 = nc.dram_tensor("W1", [L, D, F], f32, kind="ExternalInput")
    b1 = nc.dram_tensor("b1", [L, F], f32, kind="ExternalInput")
    W2 = nc.dram_tensor("W2", [L, F, D], f32, kind="ExternalInput")
    b2 = nc.dram_tensor("b2", [L, D], f32, kind="ExternalInput")
    g1 = nc.dram_tensor("g1", [L, D], f32, kind="ExternalInput")
    be1 = nc.dram_tensor("be1", [L, D], f32, kind="ExternalInput")
    g2 = nc.dram_tensor("g2", [L, D], f32, kind="ExternalInput")
    be2 = nc.dram_tensor("be2", [L, D], f32, kind="ExternalInput")

    out_h = nc.dram_tensor("out_h", [T, D], f32, kind="ExternalOutput")

    KW = H * 65
    kv_in = [nc.dram_tensor(f"kv_in_{li}", [2 * T, KW], MMDT) for li in range(L)]
    kv_out = [nc.dram_tensor(f"kv_out_{li}", [GPS * 2 * T, KW], MMDT)
              for li in range(L)]

    with tile.TileContext(nc) as tc:
        with (
            tc.tile_pool(name="consts", bufs=1) as consts,
            tc.tile_pool(name="hxp", bufs=8) as hxp,
        ):
            ident = consts.tile([P, P], f32)
            make_identity(nc, ident[:])
            ones_bf = consts.tile([P, P], MMDT)
            nc.vector.memset(ones_bf[:], 1.0)
            ones_f = consts.tile([P, P], f32)
            nc.vector.memset(ones_f[:], 1.0)
            ones128 = consts.tile([P, P], f32r)
            nc.vector.tensor_copy(out=ones128[:], in_=ones_f[:])
            eps_t = consts.tile([P, 1], f32)
            nc.vector.memset(eps_t[:], EPS)

            for _rep in range(reps):
                # ================= embedding =================
                hx = []
                with (
                    tc.tile_pool(name="emb", bufs=2) as emb,
                    tc.tile_pool(name="emb_ps", bufs=4, space="PSUM") as emb_ps,
                ):
                    idx_t = emb.tile([P, NT, 1], i32, tag="idx")
                    nc.sync.dma_start(
                        out=idx_t[:],
                        in_=x_idx[:].rearrange("(j p) o -> p j o", p=P))
                    pid_t = emb.tile([P, NT, 1], i32, tag="idx")
                    nc.sync.dma_start(
                        out=pid_t[:],
                        in_=pos_idx[:].rearrange("(j p) o -> p j o", p=P))
                    for dd in range(DD):
                        hx.append(hxp.tile([P, T], f32r, tag="hx", name=f"hx{dd}"))
                    for j in range(NT):
                        tok_g = emb.tile([P, D], f32, tag="tok")
                        nc.gpsimd.indirect_dma_start(
                            out=tok_g[:], out_offset=None, in_=tok_emb[:],
                            in_offset=bass.IndirectOffsetOnAxis(
                                ap=idx_t[:, j, :], axis=0))
                        pos_g = emb.tile([P, D], f32, tag="pos")
                        nc.gpsimd.indirect_dma_start(
                            out=pos_g[:], out_offset=None, in_=sin_table[:],
                            in_offset=bass.IndirectOffsetOnAxis(
                                ap=pid_t[:, j, :], axis=0))
                        h0 = emb.tile([P, D], f32, tag="h0")
                        nc.vector.tensor_tensor(out=h0[:], in0=tok_g[:],
                                                in1=pos_g[:], op=ALU.add)
                        for dd in range(DD):
                            tp = emb_ps.tile([P, P], f32, tag="tr")
                            nc.tensor.transpose(out=tp[:],
                                                in_=h0[:, ts(dd, P)],
                                                identity=ident[:])
                            nc.vector.tensor_copy(out=hx[dd][:, ts(j, P)],
                                                  in_=tp[:])

                # ================= layers =================
                for li in range(L):
                    with (
                        tc.tile_pool(name="sbW", bufs=2) as sbW,
                        tc.tile_pool(name="sbW3", bufs=3) as sbW3,
                        tc.tile_pool(name="sb4", bufs=4) as sb4,
                        tc.tile_pool(name="mtp", bufs=5) as mtp,
                        tc.tile_pool(name="bias", bufs=1) as bias,
                    ):
                        # --- biases / gains for this layer ---
                        bq_t = bias.tile([P, DD], f32, tag="bq")
                        nc.sync.dma_start(out=bq_t[:], in_=bq[li].rearrange("(a p) -> p a", p=P))
                        bk_t = bias.tile([P, DD], f32, tag="bk")
                        nc.sync.dma_start(out=bk_t[:], in_=bk[li].rearrange("(a p) -> p a", p=P))
                        bo_t = bias.tile([P, DD], f32, tag="bo")
                        nc.sync.dma_start(out=bo_t[:], in_=bo[li].rearrange("(a p) -> p a", p=P))
                        b2_t = bias.tile([P, DD], f32, tag="b2")
                        nc.sync.dma_start(out=b2_t[:], in_=b2[li].rearrange("(a p) -> p a", p=P))
                        b1_t = bias.tile([P, FFN], f32, tag="b1")
                        nc.sync.dma_start(out=b1_t[:], in_=b1[li].rearrange("(a p) -> p a", p=P))
                        g1_t = bias.tile([P, DD], f32, tag="g1")
                        nc.sync.dma_start(out=g1_t[:], in_=g1[li].rearrange("(a p) -> p a", p=P))
                        be1_t = bias.tile([P, DD], f32, tag="be1")
                        nc.sync.dma_start(out=be1_t[:], in_=be1[li].rearrange("(a p) -> p a", p=P))
                        g2_t = bias.tile([P, DD], f32, tag="g2")
                        nc.sync.dma_start(out=g2_t[:], in_=g2[li].rearrange("(a p) -> p a", p=P))
                        be2_t = bias.tile([P, DD], f32, tag="be2")
                        nc.sync.dma_start(out=be2_t[:], in_=be2[li].rearrange("(a p) -> p a", p=P))
                        bv_row = bias.tile([1, D], f32, tag="bvr")
                        nc.sync.dma_start(out=bv_row[:], in_=bv[li][None, :])
                        bv_row_r = bias.tile([1, D], MMDT, tag="bvrr")
                        nc.vector.tensor_copy(out=bv_row_r[:], in_=bv_row[:])

                        def load_w_r(w_ap, shape, tag, name="wr", eng=None):
                            stg = sbW3.tile(shape, f32, tag="wstage", name=name + "_stg")
                            nc.sync.dma_start(out=stg[:], in_=w_ap)
                            wr = sbW.tile(shape, f32r, tag=tag, name=name)
                            if eng == "act":
                                nc.scalar.copy(out=wr[:], in_=stg[:])
                            else:
                                nc.vector.tensor_copy(out=wr[:], in_=stg[:])
                            return wr

                        # ---------- phase A: Q/K/V projections ----------
                        with tc.tile_pool(name="psA", bufs=4, space="PSUM") as psA:
                            wk_r = load_w_r(Wk[li].rearrange("(a p) m -> p a m", p=P),
                                            [P, DD, D], "wproj")
                            k_sb = []
                            for dd in range(DD):
                                ps = psA.tile([P, T], f32, tag="pj")
                                for a in range(DD):
                                    nc.tensor.matmul(
                                        out=ps[:], lhsT=wk_r[:, a, ts(dd, P)],
                                        rhs=hx[a][:], start=(a == 0),
                                        stop=(a == DD - 1))
                                kt_ = sb4.tile([P, T], MMDT, tag="k")
                                nc.vector.tensor_scalar(
                                    out=kt_[:], in0=ps[:],
                                    scalar1=bk_t[:, dd:dd + 1], scalar2=None,
                                    op0=ALU.add)
                                k_sb.append(kt_)
                                nc.sync.dma_start(
                                    out=kv_in[li][ts(dd, P), 0:D],
                                    in_=kt_[:])
                            # V token-major: stationary hx tile, moving Wv rows
                            wv_r = load_w_r(Wv[li].rearrange("(a p) m -> p a m", p=P),
                                            [P, DD, D], "wproj")
                            # bv broadcast tile
                            bvb_ps = psA.tile([P, D], f32, tag="pj")
                            nc.tensor.matmul(out=bvb_ps[:],
                                             lhsT=ones_bf[0:1, :],
                                             rhs=bv_row_r[:],
                                             start=True, stop=True)
                            bvb = bias.tile([P, D], f32, tag="bvb")
                            nc.vector.tensor_copy(out=bvb[:], in_=bvb_ps[:])
                            for j in range(NT):
                                ps = psA.tile([P, D], f32, tag="pj")
                                for a in range(DD):
                                    nc.tensor.matmul(
                                        out=ps[:], lhsT=hx[a][:, ts(j, P)],
                                        rhs=wv_r[:, a, :], start=(a == 0),
                                        stop=(a == DD - 1))
                                vt_ = sb4.tile([P, H, 65], MMDT, tag="v")
                                nc.vector.tensor_tensor(
                                    out=vt_[:, :, 0:64],
                                    in0=ps[:].rearrange("p (h c) -> p h c", h=H),
                                    in1=bvb[:].rearrange("p (h c) -> p h c", h=H),
                                    op=ALU.add)
                                nc.vector.memset(vt_[:, :, 64:65], 1.0)
                                nc.sync.dma_start(
                                    out=kv_in[li][T + j * P:T + (j + 1) * P, :],
                                    in_=vt_[:].rearrange("p h c -> p (h c)"))
                            if no_collective:
                                # timing-only variant: replicate own K/V
                                for r in range(GPS):
                                    nc.sync.dma_start(
                                        out=kv_out[li][r * 2 * T:(r + 1) * 2 * T, :],
                                        in_=kv_in[li][:])
                            else:
                                nc.gpsimd.collective_compute(
                                    "AllGather", ALU.bypass,
                                    replica_groups=GROUPS,
                                    ins=[kv_in[li][:]], outs=[kv_out[li][:]])
                            # Q after the AG is kicked off
                            wq_r = load_w_r(Wq[li].rearrange("(a p) m -> p a m", p=P),
                                            [P, DD, D], "wproj")
                            q_sb = []
                            for dd in range(DD):
                                ps = psA.tile([P, T], f32, tag="pj")
                                for a in range(DD):
                                    nc.tensor.matmul(
                                        out=ps[:], lhsT=wq_r[:, a, ts(dd, P)],
                                        rhs=hx[a][:], start=(a == 0),
                                        stop=(a == DD - 1))
                                qt_ = sb4.tile([P, T], MMDT, tag="q")
                                nc.vector.tensor_scalar(
                                    out=qt_[:], in0=ps[:],
                                    scalar1=bq_t[:, dd:dd + 1], scalar2=None,
                                    op0=ALU.add)
                                q_sb.append(qt_)
                            wo_view = Wo[li].rearrange(
                                "(c hh p) m -> c p hh m", c=2, hh=4, p=64)
                            wo_c = []
                            for c in range(2):
                                woc = load_w_r(wo_view[c], [64, 4, D], "wproj",
                                               name=f"wo{c}")
                                wo_c.append(woc)

                        # views of the gathered K/V
                        kv_k_view = kv_out[li][:].rearrange(
                            "(r a pp p) n -> a pp p r n", r=GPS, a=2, pp=DD, p=P)
                        kv_v_view = kv_out[li][:].rearrange(
                            "(r a j p) n -> a p r j n", r=GPS, a=2, j=NT, p=P)

                        # ---------- phase B: attention ----------
                        an_sb = []
                        with tc.tile_pool(name="psB", bufs=2, space="PSUM") as psB:
                            for pp in range(PAIRS):
                                kf = sbW.tile([P, GPS, T], MMDT, tag="kfr")
                                nc.sync.dma_start(out=kf[:],
                                                  in_=kv_k_view[0, pp][:, :, 0:D])
                                kff = kf[:].rearrange("p r n -> p (r n)")
                                # V for the two heads of this pair, each
                                # augmented with a ones column: the att matmul
                                # then also produces the softmax denominator
                                # in output row 64.
                                vf = sbW.tile([P, GPS, NT, 130], MMDT, tag="vfr")
                                for r in range(GPS):
                                    nc.sync.dma_start(
                                        out=vf[:, r],
                                        in_=kv_v_view[1, :, r, :,
                                                      pp * 130:(pp + 1) * 130])
                                vff = vf[:].rearrange("p r j c -> p (r j) c")

                                att = [psB.tile([65, T], f32, tag="att",
                                                name=f"att{pp}_{hh}")
                                       for hh in range(2)]
                                for kt in range(KTN):
                                    sg = psB.tile([P, 2, T], f32, tag="sc")
                                    nc.tensor.matmul(
                                        out=sg[:, 0, :],
                                        lhsT=kff[0:64, ts(kt, P)],
                                        rhs=q_sb[pp][0:64, :],
                                        start=True, stop=True,
                                        tile_position=(0, 0))
                                    nc.tensor.matmul(
                                        out=sg[:, 1, :],
                                        lhsT=kff[64:128, ts(kt, P)],
                                        rhs=q_sb[pp][64:128, :],
                                        start=True, stop=True,
                                        tile_position=(64, 0))
                                    e_t = sbW.tile([P, 2, T], MMDT, tag="e")
                                    nc.scalar.activation(out=e_t[:], in_=sg[:],
                                                         func=AF.Exp, scale=0.125)
                                    first, last = kt == 0, kt == KTN - 1
                                    for hh in range(2):
                                        nc.tensor.matmul(
                                            out=att[hh][:],
                                            lhsT=vff[:, kt, hh * 65:(hh + 1) * 65],
                                            rhs=e_t[:, hh, :], start=first,
                                            stop=last)
                                for hh in range(2):
                                    rc1 = sbW.tile([65, T], f32, tag="rc")
                                    nc.scalar.activation(
                                        out=rc1[64:65, :], in_=att[hh][64:65, :],
                                        func=AF.Ln)
                                    rc2 = sbW.tile([65, T], MMDT, tag="rc")
                                    nc.scalar.activation(
                                        out=rc2[64:65, :], in_=rc1[64:65, :],
                                        func=AF.Exp, scale=-1.0)
                                    rb_ps = psB.tile([64, T], f32, tag="rb")
                                    nc.tensor.matmul(
                                        out=rb_ps[:],
                                        lhsT=ones_bf[64:65, 0:64],
                                        rhs=rc2[64:65, :],
                                        start=True, stop=True)
                                    rb_sb = sbW.tile([64, T], f32, tag="rb_sb")
                                    nc.vector.tensor_copy(out=rb_sb[:], in_=rb_ps[:])
                                    an = hxp.tile([64, T], f32r, tag="an",
                                                  name=f"an{pp}_{hh}")
                                    nc.vector.tensor_tensor(
                                        out=an[:], in0=att[hh][0:64, :],
                                        in1=rb_sb[:], op=ALU.mult)
                                    an_sb.append(an)

                        # ---------- phase C: O-proj, LN1, MLP, LN2 ----------
                        with (
                            tc.tile_pool(name="psC", bufs=2, space="PSUM") as psC,
                            tc.tile_pool(name="psY", bufs=4, space="PSUM") as psY,
                            tc.tile_pool(name="stp", bufs=2, space="PSUM") as stp,
                        ):
                            pools = {"st": stp, "sb4": sb4}
                            v1 = []
                            for dd in range(DD):
                                ps = psC.tile([P, T], f32, tag="mm1")
                                for h in range(H):
                                    c, hh = divmod(h, 4)
                                    nc.tensor.matmul(
                                        out=ps[:], lhsT=wo_c[c][:, hh, ts(dd, P)],
                                        rhs=an_sb[h][:], start=(h == 0),
                                        stop=(h == H - 1))
                                of = sbW.tile([P, T], f32, tag="otmp")
                                nc.vector.tensor_scalar(
                                    out=of[:], in0=ps[:],
                                    scalar1=bo_t[:, dd:dd + 1], scalar2=None,
                                    op0=ALU.add)
                                vv = sb4.tile([P, T], f32r, tag="vres")
                                nc.vector.tensor_tensor(
                                    out=vv[:], in0=of[:],
                                    in1=hx[dd][:].bitcast(f32), op=ALU.add)
                                v1.append(vv)
                            hmid = _layer_norm(nc, pools, v1, g1_t, be1_t,
                                               ones128, eps_t, sb4, "hmid")

                            # MLP in quarters of F
                            y_ps = [psY.tile([P, T], f32, tag="y", name=f"y{dd}")
                                    for dd in range(DD)]
                            for qf in range(4):
                                w1q = load_w_r(
                                    W1[li].rearrange("(a p) f -> p a f", p=P)[
                                        :, :, qf * 512:(qf + 1) * 512],
                                    [P, DD, 512], "wmlp", eng="act")
                                mts = []
                                for ffq in range(4):
                                    ff = qf * 4 + ffq
                                    ps = psC.tile([P, T], f32, tag="mm1")
                                    for a in range(DD):
                                        nc.tensor.matmul(
                                            out=ps[:],
                                            lhsT=w1q[:, a, ts(ffq, P)],
                                            rhs=hmid[a][:], start=(a == 0),
                                            stop=(a == DD - 1))
                                    mt = mtp.tile([P, T], f32r, tag="mt")
                                    nc.vector.tensor_scalar(
                                        out=mt[:], in0=ps[:],
                                        scalar1=b1_t[:, ff:ff + 1],
                                        scalar2=0.0, op0=ALU.add, op1=ALU.max)
                                    mts.append(mt)
                                w2q = load_w_r(
                                    W2[li].rearrange("(a p) m -> p a m", p=P)[
                                        :, qf * 4:(qf + 1) * 4, :],
                                    [P, 4, D], "wmlp", eng="act")
                                for dd in range(DD):
                                    for ffq in range(4):
                                        nc.tensor.matmul(
                                            out=y_ps[dd][:],
                                            lhsT=w2q[:, ffq, ts(dd, P)],
                                            rhs=mts[ffq][:],
                                            start=(qf == 0 and ffq == 0),
                                            stop=(qf == 3 and ffq == 3))
                            v2 = []
                            for dd in range(DD):
                                yf = sbW.tile([P, T], f32, tag="otmp")
                                nc.vector.tensor_scalar(
                                    out=yf[:], in0=y_ps[dd][:],
                                    scalar1=b2_t[:, dd:dd + 1], scalar2=None,
                                    op0=ALU.add)
                                vv = sb4.tile([P, T], f32r, tag="vres")
                                nc.vector.tensor_tensor(
                                    out=vv[:], in0=yf[:],
                                    in1=hmid[dd][:].bitcast(f32), op=ALU.add)
                                v2.append(vv)
                            hx = _layer_norm(nc, pools, v2, g2_t, be2_t,
                                             ones128, eps_t, hxp, "hx")

                # ================= output transpose =================
                with (
                    tc.tile_pool(name="fin", bufs=2) as fin,
                    tc.tile_pool(name="fin_ps", bufs=4, space="PSUM") as fin_ps,
                ):
                    for j in range(NT):
                        ot = fin.tile([P, D], f32, tag="ot")
                        for dd in range(DD):
                            tp = fin_ps.tile([P, P], f32, tag="tr")
                            nc.tensor.transpose(
                                out=tp[:], in_=hx[dd][:, ts(j, P)].bitcast(f32),
                                identity=ident[:])
                            nc.vector.tensor_copy(out=ot[:, ts(dd, P)], in_=tp[:])
                        nc.sync.dma_start(out=out_h[ts(j, P), :], in_=ot[:])

    nc.compile()
    return nc


_NC_CACHE = {}


def _get_nc(reps=1, no_collective=False):
    key = (reps, no_collective)
    if key not in _NC_CACHE:
        _NC_CACHE[key] = build_encoder(reps, no_collective)
    return _NC_CACHE[key]


def make_in_maps(x, tok_emb, sin_table, Wq, bq, Wk, bk, Wv, bv, Wo, bo,
                 W1, b1, W2, b2, g1, be1, g2, be2):
    x = np.asarray(x)
    shared = dict(
        tok_emb=np.ascontiguousarray(tok_emb, np.float32),
        sin_table=np.ascontiguousarray(sin_table, np.float32),
        Wq=np.ascontiguousarray(Wq, np.float32),
        Wk=np.ascontiguousarray(Wk, np.float32),
        Wv=np.ascontiguousarray(Wv, np.float32),
        Wo=np.ascontiguousarray(Wo, np.float32),
        bq=np.ascontiguousarray(bq, np.float32),
        bk=np.ascontiguousarray(bk, np.float32),
        bv=np.ascontiguousarray(bv, np.float32),
        bo=np.ascontiguousarray(bo, np.float32),
        W1=np.ascontiguousarray(W1, np.float32),
        b1=np.ascontiguousarray(b1, np.float32),
        W2=np.ascontiguousarray(W2, np.float32),
        b2=np.ascontiguousarray(b2, np.float32),
        g1=np.ascontiguousarray(g1, np.float32),
        be1=np.ascontiguousarray(be1, np.float32),
        g2=np.ascontiguousarray(g2, np.float32),
        be2=np.ascontiguousarray(be2, np.float32),
    )
    in_maps = []
    for c in range(NC):
        b, sl = divmod(c, GPS)
        xs = np.ascontiguousarray(x[b, sl * T:(sl + 1) * T].astype(np.int32))
        pos = ((np.arange(sl * T, (sl + 1) * T, dtype=np.int64) + 1)
               * (xs != 0)).astype(np.int32)
        m = dict(shared)
        m["x_idx"] = xs.reshape(T, 1)
        m["pos_idx"] = pos.reshape(T, 1)
        in_maps.append(m)
    return in_maps


def kernel(x, mask, tok_emb, sin_table, Wq, bq, Wk, bk, Wv, bv, Wo, bo,
           W1, b1, W2, b2, g1, be1, g2, be2):
    # mask is all-ones per the problem spec (fill="ones"); softmax with an
    # all-true mask is plain softmax, so it is not shipped to the device.
    nc = _get_nc(1)
    in_maps = make_in_maps(x, tok_emb, sin_table, Wq, bq, Wk, bk, Wv, bv,
                           Wo, bo, W1, b1, W2, b2, g1, be1, g2, be2)
    res = run_bass_kernel_spmd(nc, in_maps, core_ids=list(range(NC)))
    out = np.zeros((B, S, D), np.float32)
    for c in range(NC):
        b, sl = divmod(c, GPS)
        out[b, sl * T:(sl + 1) * T, :] = res.results[c]["out_h"]
    return out
